# revision 25
# baseline (speedup 1.0000x reference)
"""GQA attention (B=2,S=2048,E=2048,H=32,KVH=8,D=64, RoPE, non-causal) on 8 TRN2 cores.

Sharding: core = 4*b + g  (b = batch, g = head-group).  Each core owns one batch
and 8 q-heads / 2 kv-heads, computes a partial output projection; host sums the
4 group partials per batch.

v3: every adjacent PE matmul pair uses disjoint row groups (rows 0-63 vs
64-127) so LDWEIGHTS pulls ahead and matmuls overlap in the array:
  - scores for a head pair go into ONE [128,1024] PSUM tile ({A|B} halves),
    one exp per k-chunk covers both heads, so the two score matmuls become
    ready together and issue back-to-back with alternating row groups
  - PV and all projections split their 128-deep contraction into two 64-row
    matmuls accumulating into the same PSUM region (row-group alternation)
  - PV matmuls of the previous pair issue one k-chunk per step right before
    the score matmuls (matches pt pool reuse, deadlock-free by construction)
  - softmax denominators: ONE [128,512] reciprocal per 4 heads; broadcast via
    a 1-row matmul at 32-aligned bases; normalization multiplies on DVE
  - input DMAs spread across sync/vector/gpsimd/scalar queues, tables first
"""

import numpy as np
import ml_dtypes

import concourse.bass as bass
import concourse.tile as tile
from concourse import bacc, mybir
from concourse.bass_utils import run_bass_kernel_spmd

BF16 = ml_dtypes.bfloat16
F32 = mybir.dt.float32
BF = mybir.dt.bfloat16

B, S, E = 2, 2048, 2048
H, KVH, D = 32, 8, 64
N_CORES = 8
FH = 512          # features per core (8 heads * 64)
EC, SC = 16, 4    # e-chunks(128), s-chunks(512)

_CACHE = {}

_OHOT = np.zeros((8, 512), dtype=BF16)
for _h in range(8):
    _OHOT[_h, 64 * _h:64 * (_h + 1)] = 1.0


def _build():
    nc = bacc.Bacc("TRN2", target_bir_lowering=False, debug=False,
                   num_devices=N_CORES)
    xt_d = nc.dram_tensor("xt", [E, S], BF, kind="ExternalInput")
    wqt_d = nc.dram_tensor("wqt", [E, FH], BF, kind="ExternalInput")
    wkt_d = nc.dram_tensor("wkt", [E, 128], BF, kind="ExternalInput")
    wvt_d = nc.dram_tensor("wvt", [E, 128], BF, kind="ExternalInput")
    wot_d = nc.dram_tensor("wot", [FH, E], BF, kind="ExternalInput")
    cos_d = nc.dram_tensor("cost", [128, S], BF, kind="ExternalInput")
    nsin_d = nc.dram_tensor("nsint", [128, S], BF, kind="ExternalInput")
    ohot_d = nc.dram_tensor("ohot", [8, 512], BF, kind="ExternalInput")
    out_d = nc.dram_tensor("out", [S, E], BF, kind="ExternalOutput")

    from contextlib import ExitStack
    with ExitStack() as ctx:
        tc = ctx.enter_context(tile.TileContext(nc))
        pool = lambda *a, **k: ctx.enter_context(tc.tile_pool(*a, **k))
        xt_p = pool(name="xt", bufs=16)
        w_p = pool(name="w", bufs=16)        # wqt early, wot late (shared slots)
        wkv_p = pool(name="wkv", bufs=32)
        cs_p = pool(name="cs", bufs=2)
        raw_p = pool(name="raw", bufs=1)
        t2_p = pool(name="t2", bufs=2)
        qkt_p = pool(name="qkt", bufs=6)
        v_p = pool(name="v", bufs=32)
        pt_p = pool(name="pt", bufs=16)
        pvs_p = pool(name="pvs", bufs=10)
        attnt_p = pool(name="attnt", bufs=8)
        dcol_p = pool(name="dcol", bufs=4)
        rec_p = pool(name="rec", bufs=2)
        nt_p = pool(name="nt", bufs=2)
        ostage_p = pool(name="ostage", bufs=2)
        sc_p = pool(name="sc", bufs=2, space="PSUM")
        pv_p = pool(name="pv", bufs=2, space="PSUM")
        mm_p = pool(name="mm", bufs=2, space="PSUM")

        EXP = mybir.ActivationFunctionType.Exp
        LO, HI = slice(0, 64), slice(64, 128)

        # ---- input loads (spread across engine DMA queues; tables first) ----
        cos_t = cs_p.tile([128, S], BF, tag="cs")
        nc.gpsimd.dma_start(cos_t[:], cos_d[:, :])
        nsin_t = cs_p.tile([128, S], BF, tag="cs")
        nc.gpsimd.dma_start(nsin_t[:], nsin_d[:, :])
        # issue order matters: K proj consumes wkt + xt in ec order, so
        # those go first on each queue; wqt/wvt follow (needed later)
        wkt = []
        for i in range(EC):
            t = wkv_p.tile([128, 128], BF, tag="wkv", name="wkt")
            nc.gpsimd.dma_start(t[:], wkt_d[128 * i:128 * (i + 1), :])
            wkt.append(t)
        xt = [None] * EC
        for i in range(0, EC, 2):
            t = xt_p.tile([128, S], BF, tag="xt", name="xte")
            nc.sync.dma_start(t[:], xt_d[128 * i:128 * (i + 1), :])
            xt[i] = t
        for i in range(1, EC, 2):
            t = xt_p.tile([128, S], BF, tag="xt", name="xto")
            nc.gpsimd.dma_start(t[:], xt_d[128 * i:128 * (i + 1), :])
            xt[i] = t
        wqt = []
        for i in range(EC):
            t = w_p.tile([128, FH], BF, tag="w", name="wqt")
            nc.gpsimd.dma_start(t[:], wqt_d[128 * i:128 * (i + 1), :])
            wqt.append(t)
        wvt = []
        for i in range(EC):
            t = wkv_p.tile([128, 128], BF, tag="wkv", name="wvt")
            nc.gpsimd.dma_start(t[:], wvt_d[128 * i:128 * (i + 1), :])
            wvt.append(t)
        # one-hot weights for denominator broadcast: row h of slice
        # [0:8, 64h:64h+64] is ones -> matmul copies rec row h to 64 partitions
        ohot = cs_p.tile([8, 512], BF, tag="ohot", bufs=1)
        nc.gpsimd.dma_start(ohot[:], ohot_d[:, :])

        def rope(src, dst):
            # dst = src*cos + shift32(src)*nsin, per 64-row head block.
            # partition shift must go through DMA (engines are lane-locked)
            qs = t2_p.tile([128, S], BF, tag="t2", name="qs")
            for blk in (0, 64):
                nc.sync.dma_start(qs[blk:blk + 32, :],
                                  src[blk + 32:blk + 64, :])
                nc.sync.dma_start(qs[blk + 32:blk + 64, :],
                                  src[blk:blk + 32, :])
            t2 = t2_p.tile([128, S], BF, tag="t2", name="t2")
            nc.vector.tensor_mul(t2[:], qs[:], nsin_t[:])
            nc.vector.tensor_mul(dst[:], src[:], cos_t[:])
            nc.vector.tensor_add(dst[:], dst[:], t2[:])

        SPLIT = False  # row-split accumulation hangs HW; keep single MMs

        def acc_mm(ps, lhsT, rhs, first, last):
            if not SPLIT:
                nc.tensor.matmul(ps, lhsT, rhs, start=first, stop=last)
                return
            # one 128-deep contraction step as two row-group-alternating
            # 64-deep matmuls accumulating into the same psum region
            nc.tensor.matmul(ps, lhsT[LO, :], rhs[LO, :],
                             start=first, stop=False)
            nc.tensor.matmul(ps, lhsT[HI, :], rhs[HI, :],
                             start=False, stop=last)

        # ---- K projection + rope (prologue, serial) ----
        # ec-outer so matmuls chase the arriving xt DMAs (no bulk stall);
        # four open accumulation slices across two (otherwise idle) sc tiles
        kraw = raw_p.tile([128, S], BF, tag="raw", name="kraw")
        kps = [sc_p.tile([128, 1024], F32, tag="sc", name="kps")
               for _ in range(2)]
        for ec in range(EC):
            for sc in range(SC):
                nc.tensor.matmul(kps[sc // 2][:, 512 * (sc % 2):512 * (sc % 2 + 1)],
                                 wkt[ec][:, 0:128],
                                 xt[ec][:, 512 * sc:512 * (sc + 1)],
                                 start=(ec == 0), stop=(ec == EC - 1))
        for sc in range(SC):
            nc.vector.tensor_copy(kraw[:, 512 * sc:512 * (sc + 1)],
                                  kps[sc // 2][:, 512 * (sc % 2):512 * (sc % 2 + 1)])
        rope(kraw, kraw)
        # duplicate each kv head across both partition halves so paired score
        # matmuls can use disjoint PE row groups
        ktd = [qkt_p.tile([128, S], BF, tag="qkt", name=f"ktd{i}")
               for i in range(2)]
        for kv in range(2):
            src = kraw[64 * kv:64 * (kv + 1), :]
            nc.sync.dma_start(ktd[kv][0:64, :], src)
            nc.sync.dma_start(ktd[kv][64:128, :], src)

        qt = [None] * 4
        qps = {}  # fc -> in-flight psum tile for Q projection

        def q_proj_mm(fc, sc, ec0):
            if ec0 == 0:
                qps[fc] = mm_p.tile([128, 512], F32, tag="mm", name="qps")
            ps = qps[fc]
            for ec in range(ec0, ec0 + 4):
                acc_mm(ps[:], wqt[ec][:, 128 * fc:128 * (fc + 1)],
                       xt[ec][:, 512 * sc:512 * (sc + 1)],
                       ec == 0, ec == EC - 1)

        def q_copy(fc, sc, qraw):
            nc.vector.tensor_copy(qraw[:, 512 * sc:512 * (sc + 1)], qps[fc])

        def q_finish(fc, qraw):
            qf = qkt_p.tile([128, S], BF, tag="qkt", name=f"qt{fc}")
            rope(qraw, qf)
            qt[fc] = qf

        # Q head-pairs 0 and 1 in the prologue so attention can start
        for fc in (0, 1):
            qraw = raw_p.tile([128, S], BF, tag="raw", name=f"qraw{fc}")
            for sc in range(SC):
                for ec0 in (0, 4, 8, 12):
                    q_proj_mm(fc, sc, ec0)
                q_copy(fc, sc, qraw)
            q_finish(fc, qraw)

        # ---- deferred fill work: (cost_ns, closure), consumed between
        # score matmuls by a per-step budget ----
        fill_q = []

        def make_q_closures(fc):
            qraw = [None]

            def start():
                qraw[0] = raw_p.tile([128, S], BF, tag="raw", name=f"qraw{fc}")

            fill_q.append((0, start))
            for sc in range(SC):
                for ec0 in (0, 4, 8, 12):
                    fill_q.append((500, lambda fc=fc, sc=sc, ec0=ec0:
                                   q_proj_mm(fc, sc, ec0)))
                fill_q.append((100, lambda fc=fc, sc=sc: q_copy(fc, sc, qraw[0])))
            fill_q.append((100, lambda fc=fc: q_finish(fc, qraw[0])))

        vt = {}  # (kc, kv_local) -> [128, 65]
        vps = {}

        def v_proj_mm(kc, ec0):
            if ec0 == 0:
                vps[kc] = mm_p.tile([128, 128], F32, tag="mm", name="vps")
            ps = vps[kc]
            for ec in range(ec0, ec0 + 4):
                acc_mm(ps[:], xt[ec][:, 128 * kc:128 * (kc + 1)],
                       wvt[ec][:, 0:128], ec == 0, ec == EC - 1)

        def v_copy(kc):
            ps = vps.pop(kc)
            for kv in range(2):
                v = v_p.tile([128, 65], BF, tag="v")
                nc.vector.tensor_copy(v[:, 0:64], ps[:, 64 * kv:64 * (kv + 1)])
                nc.gpsimd.memset(v[:, 64:65], 1.0)
                vt[(kc, kv)] = v

        for kc in range(16):
            for ec0 in (0, 4, 8, 12):
                fill_q.append((180, lambda kc=kc, ec0=ec0: v_proj_mm(kc, ec0)))
            fill_q.append((100, lambda kc=kc: v_copy(kc)))
        make_q_closures(1)
        make_q_closures(2)
        make_q_closures(3)

        def drain(budget):
            while fill_q:
                cost, fn = fill_q.pop(0)
                fn()
                budget -= cost
                if budget <= 0:
                    return

        # ---- attention + output projection ----
        pv_queue = []
        pvstash = {}
        attnt = {}  # (fc, qc) -> [128, 512] bf16

        wot = {}
        for fc2 in range(4):
            for ecb in range(4):
                t = w_p.tile([128, 512], BF, tag="w", name="wot")
                nc.sync.dma_start(
                    t[:], wot_d[128 * fc2:128 * (fc2 + 1),
                                512 * ecb:512 * (ecb + 1)])
                wot[(fc2, ecb)] = t

        # denominator collect tiles: 8 heads at partitions 0-7
        dcol = {}

        def alloc_dcol(qc):
            dcol[qc] = dcol_p.tile([8, 512], BF, tag="dcol",
                                   name=f"dcol{qc}")

        def make_pv_closures(qc, p, pts):
            kv = p // 2
            pv = {}

            def pv_mm(kc):
                if kc == 0:
                    pv['A'] = pv_p.tile([65, 512], F32, tag="pv", name="pvA")
                    pv['B'] = pv_p.tile([65, 512], F32, tag="pv", name="pvB")
                va = vt[(kc, kv)]
                first, last = (kc == 0), (kc == 15)
                pa, pb = pts[kc][:, 0:512], pts[kc][:, 512:1024]
                nc.tensor.matmul(pv['A'][:], va[:, 0:65], pa[:],
                                 start=first, stop=last)
                nc.tensor.matmul(pv['B'][:], va[:, 0:65], pb[:],
                                 start=first, stop=last)
                if kc == 15:
                    # stage PV to SBUF (frees PSUM) + collect denominators
                    for side, h in (('A', 2 * p), ('B', 2 * p + 1)):
                        pvs = pvs_p.tile([65, 512], BF, tag="pvs")
                        nc.vector.tensor_copy(pvs[:], pv[side][0:65, :])
                        nc.sync.dma_start(dcol[qc][h:h + 1, :],
                                          pvs[64:65, :])
                        pvstash[(qc, h)] = pvs

            for kc in range(16):
                pv_queue.append(lambda kc=kc: pv_mm(kc))

        def norm_qc(qc):
            rec = rec_p.tile([8, 512], F32, tag="rec")
            nc.vector.reciprocal(rec[0:8, :], dcol[qc][0:8, :])
            recb = rec_p.tile([8, 512], BF, tag="recb")
            nc.vector.tensor_copy(recb[0:8, :], rec[0:8, :])
            for h in range(8):
                fc, hl = h // 2, h % 2
                bc = mm_p.tile([64, 512], F32, tag="mm", name="bc")
                nc.tensor.matmul(bc[:], ohot[0:8, 64 * h:64 * (h + 1)],
                                 recb[0:8, :], start=True, stop=True)
                pvs = pvstash.pop((qc, h))
                if hl == 0:
                    at = attnt_p.tile([128, 512], BF, tag="attnt")
                    attnt[(fc, qc)] = at
                    nc.vector.tensor_mul(at[0:64, :], pvs[0:64, :], bc[:])
                else:
                    nt = nt_p.tile([64, 512], BF, tag="nt")
                    nc.vector.tensor_mul(nt[:], pvs[0:64, :], bc[:])
                    nc.sync.dma_start(attnt[(fc, qc)][64:128, :], nt[:])

        def make_outproj_closures(qc):
            def op_mm(stl, ecb):
                st = 4 * qc + stl
                op = mm_p.tile([128, 512], F32, tag="mm", name="op")
                for fc2 in range(4):
                    acc_mm(op[:],
                           attnt[(fc2, qc)][:, 128 * stl:128 * (stl + 1)],
                           wot[(fc2, ecb)][:], fc2 == 0, fc2 == 3)
                so = ostage_p.tile([128, 512], BF, tag="ostage")
                nc.vector.tensor_copy(so[:], op[:])
                nc.gpsimd.dma_start(
                    out_d[128 * st:128 * (st + 1),
                          512 * ecb:512 * (ecb + 1)], so[:])

            for stl in range(4):
                for ecb in range(4):
                    fill_q.append((950, lambda stl=stl, ecb=ecb: op_mm(stl, ecb)))

        norm_due = []

        # pre-issue ~12us of V-projection work so the PE has queued matmuls
        # to execute while DVE/DMA run the rope chains before first scores
        drain(6500)

        for qc in range(SC):
            qsl = slice(512 * qc, 512 * (qc + 1))
            alloc_dcol(qc)
            for p in range(4):
                kv = p // 2
                while qt[p] is None:
                    fill_q.pop(0)[1]()
                if qc == 0 and p == 1:
                    while len(vt) < 32:
                        fill_q.pop(0)[1]()
                pts = []
                for kc in range(16):
                    # previous pair's PV for this kc (matches pt reuse)
                    if pv_queue:
                        pv_queue.pop(0)()
                    sp = sc_p.tile([128, 1024], F32, tag="sc", name="sp")
                    ksl = slice(128 * kc, 128 * (kc + 1))
                    # {A|B} halves: same readiness, alternating row groups
                    nc.tensor.matmul(sp[:, 0:512], ktd[kv][LO, ksl],
                                     qt[p][LO, qsl], start=True, stop=True)
                    nc.tensor.matmul(sp[:, 512:1024], ktd[kv][HI, ksl],
                                     qt[p][HI, qsl], start=True, stop=True)
                    drain(1300 if qc == 0 else 500)
                    pt = pt_p.tile([128, 1024], BF, tag="pt", name="pt")
                    nc.scalar.activation(pt[:], sp[:], EXP, 0.0, 0.125)
                    pts.append(pt)
                make_pv_closures(qc, p, pts)
                if norm_due and (qc, p) >= (norm_due[0] + 1, 1):
                    dqc = norm_due.pop(0)
                    fill_q.append((0, lambda dqc=dqc: norm_qc(dqc)))
                    make_outproj_closures(dqc)
            norm_due.append(qc)
        # tail: flush remaining PV work, then norms + output projections
        while pv_queue:
            pv_queue.pop(0)()
        while norm_due:
            dqc = norm_due.pop(0)
            fill_q.append((0, lambda dqc=dqc: norm_qc(dqc)))
            make_outproj_closures(dqc)
        drain(10 ** 9)

    nc.compile()
    return nc


def _tables():
    inv = 1.0 / (10000.0 ** (np.arange(0, 64, 2, dtype=np.float64) / 64))
    t = np.arange(S, dtype=np.float64)
    emb = np.concatenate([np.outer(t, inv)] * 2, -1)          # [S,64]
    cos_t = np.cos(emb).T.astype(np.float32)                  # [64,S]
    sin_t = np.sin(emb).T.astype(np.float32)
    ssin = np.concatenate([-sin_t[:32], sin_t[32:]], 0)
    cos_tile = np.ascontiguousarray(np.vstack([cos_t, cos_t])).astype(BF16)
    nsin_tile = np.ascontiguousarray(np.vstack([ssin, ssin])).astype(BF16)
    return cos_tile, nsin_tile


def kernel(x, Wq, Wk, Wv, Wo):
    x = np.asarray(x, np.float32)
    Wq, Wk, Wv, Wo = (np.asarray(w, np.float32) for w in (Wq, Wk, Wv, Wo))
    if "nc" not in _CACHE:
        _CACHE["nc"] = _build()
    nc = _CACHE["nc"]
    cos_tile, nsin_tile = _tables()
    xts = [np.ascontiguousarray(x[b].T).astype(BF16) for b in range(B)]
    in_maps = []
    for core in range(N_CORES):
        b, g = divmod(core, 4)
        fsl = slice(FH * g, FH * (g + 1))
        dsl = slice(128 * g, 128 * (g + 1))
        in_maps.append({
            "xt": xts[b],
            "wqt": np.ascontiguousarray(Wq[fsl].T).astype(BF16),
            "wkt": np.ascontiguousarray(Wk[dsl].T).astype(BF16),
            "wvt": np.ascontiguousarray(Wv[dsl].T).astype(BF16),
            "wot": np.ascontiguousarray(Wo[:, fsl].T).astype(BF16),
            "cost": cos_tile,
            "nsint": nsin_tile,
            "ohot": _OHOT,
        })
    res = run_bass_kernel_spmd(nc, in_maps, core_ids=list(range(N_CORES)),
                               **_CACHE.get("run_kwargs", {}))
    _CACHE["last_result"] = res
    out = np.empty((B, S, E), np.float32)
    for b in range(B):
        out[b] = sum(res.results[4 * b + g]["out"].astype(np.float32)
                     for g in range(4))
    return out
